# revision 9
# baseline (speedup 1.0000x reference)
"""Trainium2 Bass kernel for nn_ContrastiveLoss_rec (8-core data-parallel).

Math (per reference):
    wA_is = A_is @ W.T + b ; wA_em = A_em @ W.T + b
    diag_is = sum((0.4*m + 0.6*tr_m) * wA_is, -1)
    diag_em = sum((0.4*m + 0.6*tr_m) * wA_em, -1)
    loss = sum(max(0.2 + diag_is - diag_em, 0))

Algebraic simplification used here:
    mc  = 0.4*m + 0.6*tr_m          (bias b cancels in the difference)
    z   = rowdot(mc, (A_is - A_em) @ W.T)
        = rowdot(D, mc @ W)          with D = A_is - A_em
    loss = sum(max(0.2 + z, 0))
Folding the 0.6:  mc = 0.6*(tr_m + (2/3) m) = 0.6*mc'
    loss = 0.6 * sum(max(z' + 1/3, 0)),  z' = rowdot(D, mc' @ W)

Per-core plan (B_loc = 1024 rows):
  - DMA shards of the four [B,E] tensors + replicated W.
  - PE transposes mc' chunks directly from m/tr_m with a scaled-identity
    accumulate:  psum = tr_m_chunk.T @ I + m_chunk.T @ ((2/3) I).
  - Main matmul P = mc' @ W in float32r (full-rate fp32 path).
  - DVE fused tensor_tensor_reduce: z'_partial = sum(D * P, free-axis).
  - Hinge + row reduce, partition reduce via matmul with a 0.6-filled ones
    vector, scalar out per core; host sums the 8 partials.
"""

import numpy as np

import concourse.bass as bass
import concourse.mybir as mybir
import concourse.tile as tile
from concourse.bass_utils import run_bass_kernel_spmd

N_CORES = 8
B, E = 8192, 1024
B_LOC = B // N_CORES          # 1024 rows per core
P = 128                       # partitions
NBT = B_LOC // P              # 8 b-tiles per core
ST = 2                        # b-tiles per DMA super-tile (1 MiB DMAs)
NST = NBT // ST               # 4 super-tiles
KT = E // P                   # 8 contraction chunks
NF = 512                      # matmul moving free dim (one PSUM bank fp32)
NCH = E // NF                 # 2 n-chunks

F32 = mybir.dt.float32
F32R = mybir.dt.float32r
AX = mybir.AluOpType


def _make_scaled_identity(nc, ap, val):
    nc.gpsimd.memset(ap, 0.0)
    nc.gpsimd.affine_select(
        out=ap,
        in_=ap,
        compare_op=AX.not_equal,
        fill=float(val),
        base=0,
        pattern=[[-1, ap.shape[1]]],
        channel_multiplier=1,
    )


def build(st=ST, io_bufs=3, dma_engines=("sync",), repeat=1):
    """Build the single-core Bass program (SPMD across 8 cores)."""
    nst = NBT // st
    nc = bass.Bass(
        "TRN2", target_bir_lowering=False, debug=False, num_devices=N_CORES
    )

    A_is = nc.dram_tensor("a_is", [B_LOC, E], F32, kind="ExternalInput").ap()
    A_em = nc.dram_tensor("a_em", [B_LOC, E], F32, kind="ExternalInput").ap()
    M_in = nc.dram_tensor("m_in", [B_LOC, E], F32, kind="ExternalInput").ap()
    TR_m = nc.dram_tensor("tr_m", [B_LOC, E], F32, kind="ExternalInput").ap()
    W_in = nc.dram_tensor("w_in", [E, E], F32, kind="ExternalInput").ap()
    OUT = nc.dram_tensor("out", [1, 1], F32, kind="ExternalOutput").ap()

    _dma_idx = [0]

    def dma(dst, src):
        eng = getattr(nc, dma_engines[_dma_idx[0] % len(dma_engines)])
        _dma_idx[0] += 1
        eng.dma_start(dst, src)

    with tile.TileContext(nc) as tc:
        with (
            tc.tile_pool(name="const", bufs=1) as cpool,
            tc.tile_pool(name="wpool", bufs=1) as wpool,
            tc.tile_pool(name="io", bufs=io_bufs) as iopool,
            tc.tile_pool(name="dbuf", bufs=2) as dpool,
            tc.tile_pool(name="mct", bufs=2) as mctpool,
            tc.tile_pool(name="ttr", bufs=2) as ttrpool,
            tc.tile_pool(name="acc", bufs=1) as accpool,
            tc.tile_pool(name="ps_t", bufs=4, space="PSUM") as pst,
            tc.tile_pool(name="ps_mm", bufs=2, space="PSUM") as psmm,
            tc.tile_pool(name="ps_fin", bufs=1, space="PSUM") as psfin,
        ):
            ident1 = cpool.tile([P, P], F32)
            _make_scaled_identity(nc, ident1[:], 1.0)
            ones06 = cpool.tile([P, 1], F32)
            nc.vector.memset(ones06[:], 0.6)

            for _rep in range(repeat):
                # z' partials: one column per (b-tile, n-chunk)
                zacc = accpool.tile([P, NBT * NCH], F32, tag="zacc")

                # Replicated weight: [e_part, k_chunk, e'] layout.
                # SWDGE DMA casts fp32 -> fp32r (rounding) during the load.
                w_sb = wpool.tile([P, KT, E], F32R, tag="w")
                nc.gpsimd.dma_start(
                    w_sb[:], W_in.rearrange("(ko p) n -> p ko n", p=P)
                )

                for s in range(nst):
                    rows = bass.ds(s * st * P, st * P)

                    m_t = iopool.tile([P, st, E], F32, tag="m")
                    trm_t = iopool.tile([P, st, E], F32, tag="trm")
                    ais_t = iopool.tile([P, st, E], F32, tag="ais")
                    aem_t = iopool.tile([P, st, E], F32, tag="aem")
                    dma(m_t[:], M_in[rows, :].rearrange("(t p) e -> p t e", p=P))
                    dma(trm_t[:], TR_m[rows, :].rearrange("(t p) e -> p t e", p=P))
                    dma(ais_t[:], A_is[rows, :].rearrange("(t p) e -> p t e", p=P))
                    dma(aem_t[:], A_em[rows, :].rearrange("(t p) e -> p t e", p=P))

                    # D = A_is - A_em  (natural layout)
                    d_t = dpool.tile([P, st, E], F32, tag="d")
                    nc.vector.tensor_tensor(
                        d_t[:], ais_t[:], aem_t[:], AX.subtract
                    )

                    # mc' = (2/3)*m + tr_m  (scale on ScalarE, add on VectorE)
                    mc_t = dpool.tile([P, st, E], F32, tag="mc")
                    nc.scalar.mul(mc_t[:], m_t[:], 2.0 / 3.0)
                    nc.vector.tensor_tensor(mc_t[:], mc_t[:], trm_t[:], AX.add)

                    for t in range(st):
                        # mc'^T chunks via PE identity transpose (plain fp32);
                        # the PSUM->SBUF copy rounds to fp32r for the matmul.
                        mct_t = mctpool.tile([P, KT, P], F32R, tag="mct")
                        for g in range(KT // 4):
                            pt = pst.tile([P, 4, P], F32, tag="pt")
                            for j4 in range(4):
                                j = g * 4 + j4
                                cols = bass.ds(j * P, P)
                                nc.tensor.matmul(
                                    pt[:, j4],
                                    mc_t[:, t, cols],
                                    ident1[:],
                                    is_transpose=True,
                                    start=True,
                                    stop=True,
                                )
                            nc.vector.tensor_copy(
                                mct_t[:, bass.ds(g * 4, 4), :], pt[:]
                            )

                        # P = mc' @ W  (float32r full-rate), then fused rowdot
                        for n in range(NCH):
                            ncols = bass.ds(n * NF, NF)
                            pm = psmm.tile([P, NF], F32, tag="pm")
                            for k in range(KT):
                                nc.tensor.matmul(
                                    pm[:],
                                    mct_t[:, k, :],
                                    w_sb[:, k, ncols],
                                    start=(k == 0),
                                    stop=(k == KT - 1),
                                )
                            ttr_out = ttrpool.tile([P, NF], F32, tag="ttro")
                            zi = (s * st + t) * NCH + n
                            nc.vector.scalar_tensor_tensor(
                                out=ttr_out[:],
                                in0=pm[:],
                                scalar=1.0,
                                in1=d_t[:, t, ncols],
                                op0=AX.mult,
                                op1=AX.mult,
                                accum_out=zacc[:, zi : zi + 1],
                            )

                # z'_b = sum of its n-chunk partials; hinge; row-reduce
                zrow = accpool.tile([P, NBT], F32, tag="zrow")
                nc.vector.tensor_tensor(
                    zrow[:],
                    zacc[:].rearrange("p (b n) -> p b n", n=NCH)[:, :, 0],
                    zacc[:].rearrange("p (b n) -> p b n", n=NCH)[:, :, 1],
                    AX.add,
                )
                hrow = accpool.tile([P, NBT], F32, tag="hrow")
                nc.vector.tensor_scalar(
                    hrow[:], zrow[:], 1.0 / 3.0, 0.0, AX.add, AX.max
                )
                hsum = accpool.tile([P, 1], F32, tag="hsum")
                nc.vector.reduce_sum(hsum[:], hrow[:], axis=mybir.AxisListType.X)

                # partition reduce (x0.6 folded into the ones vector)
                fin = psfin.tile([1, 1], F32, tag="fin")
                nc.tensor.matmul(fin[:], hsum[:], ones06[:], start=True, stop=True)
                out_sb = accpool.tile([1, 1], F32, tag="osb")
                nc.any.tensor_copy(out_sb[:], fin[:])
                dma(OUT[:], out_sb[:])

    return nc


def _split_multi_waits(raw: bytes) -> bytes:
    """Split multi-wait instructions into single-wait Drain carriers +
    original: this walrus build allows only one sync wait per instruction."""
    import json as _json

    d = _json.loads(raw)
    for fn in d["functions"]:
        for bb in fn["blocks"]:
            out = []
            for inst in bb["instructions"]:
                si = inst.get("sync_info") or {}
                waits = si.get("on_wait") or []
                if len(waits) > 1:
                    for i, w in enumerate(waits[:-1]):
                        carrier = {
                            "engine": inst["engine"],
                            "ins": [],
                            "name": f"{inst['name']}-sw{i}",
                            "opcode": "Drain",
                            "outs": [],
                            "sync_info": {"on_update": [], "on_wait": [w]},
                        }
                        if "debug" in inst:
                            carrier["debug"] = inst["debug"]
                        out.append(carrier)
                    inst["sync_info"] = {
                        "on_update": si.get("on_update") or [],
                        "on_wait": [waits[-1]],
                    }
                out.append(inst)
            bb["instructions"] = out
    return _json.dumps(d).encode()


def _patch_nc(nc):
    patched = _split_multi_waits(nc.to_json_bytes())
    nc.to_json_bytes = lambda: patched
    return nc


_NC_CACHE = None


def _get_nc():
    global _NC_CACHE
    if _NC_CACHE is None:
        _NC_CACHE = _patch_nc(build())
    return _NC_CACHE


def _in_maps(inputs):
    a_is = np.ascontiguousarray(np.asarray(inputs["A_is_t"], dtype=np.float32))
    a_em = np.ascontiguousarray(np.asarray(inputs["A_em_t"], dtype=np.float32))
    m = np.ascontiguousarray(np.asarray(inputs["m"], dtype=np.float32))
    tr_m = np.ascontiguousarray(np.asarray(inputs["tr_m"], dtype=np.float32))
    w = np.ascontiguousarray(np.asarray(inputs["W"], dtype=np.float32))
    maps = []
    for c in range(N_CORES):
        sl = slice(c * B_LOC, (c + 1) * B_LOC)
        maps.append(
            {
                "a_is": a_is[sl],
                "a_em": a_em[sl],
                "m_in": m[sl],
                "tr_m": tr_m[sl],
                "w_in": w,
            }
        )
    return maps


def run(inputs, trace=False, **kw):
    """Run on all 8 cores; returns (full_output, BassKernelResults)."""
    nc = _get_nc()
    res = run_bass_kernel_spmd(
        nc, _in_maps(inputs), list(range(N_CORES)), trace=trace, **kw
    )
    total = float(sum(np.float32(r["out"][0, 0]) for r in res.results))
    return np.array([total], dtype=np.float32), res


def kernel(**inputs) -> np.ndarray:
    out, _ = run(inputs, trace=False)
    return out
